# revision 72
# baseline (speedup 1.0000x reference)
"""HGRNBitMLP (BitNet-style SwiGLU MLP) on 8 TRN2 NeuronCores.

Tensor-parallel: core c owns gate channels [c*1024,(c+1)*1024) and the
matching v channels of w_gate, plus the matching w_down^T rows. Each
core ternarizes only its weight shard (no weight AllGather). The
int8-grid activations xq^T (exact integers in bf16) are AllGathered so
every core streams all 4096 tokens through its channel shard; per-token
amax rides a tiny second AllGather. h is kept f32 (spilled to DRAM).
mm2 partials are scaled per token, cast to bf16, and ReduceScattered
per 512-col H chunk.

Tokens are processed in two 2048-token halves: while half B runs mm1,
half A's tiny stat AllReduces + h-quant complete, so mm2(A) starts the
moment mm1(B) leaves the PE. Output ownership is 2x256 tokens per core;
the host gather reorders.

Ternary weights are materialized as {-2,0,+2} via two Sign activations
(scalar engine) + add, with m/254 folded into the host-computed scale
constants. The weight-quant scalars (threshold, mean|w|) are computed
host-side in f64 and passed as a 4-float input.
"""
import sys

try:
    import concourse  # noqa: F401
except ImportError:
    sys.path.insert(0, "/opt/trn_rl_repo")

import numpy as np

import concourse.tile as tile
from concourse import bacc, mybir
from concourse.bass_utils import run_bass_kernel_spmd
from concourse.masks import make_identity

F32, BF16 = mybir.dt.float32, mybir.dt.bfloat16
Alu = mybir.AluOpType
Act = mybir.ActivationFunctionType
X = mybir.AxisListType.X

NC_N = 8
B, S, H, I = 2, 2048, 2048, 8192
TOK = B * S          # 4096 tokens
TPC = TOK // NC_N    # 512 tokens/core quantized locally
TT = TPC // 128      # 4 token tiles in own shard
HK = H // 128        # 16 contraction tiles for mm1
OL = 2 * I // NC_N   # 2048 local output channels of w_gate (gate+v)
OLP = OL // 2 // 128  # 8 gate/v pair tiles
IL = I // NC_N       # 1024 local intermediate channels
ILK = IL // 128      # 8 contraction tiles for mm2
TGN = TOK // 512     # 8 token groups of 512
TGH = TGN // 2       # 4 token groups per half
TTG = 512 // 128     # 4 t-tiles per group
TTH = 2048 // 128    # 16 t-tiles per half
EPS = 1e-5
C_MAGIC = 12582912.0  # 1.5*2^23; (x+C)-C rounds f32 to nearest-even int


def build(nc):
    x_ap = nc.dram_tensor("x", [TPC, H], F32, kind="ExternalInput").ap()
    wg_ap = nc.dram_tensor("wgt", [H, OL], F32, kind="ExternalInput").ap()
    wd_ap = nc.dram_tensor("wdt", [IL, H], F32, kind="ExternalInput").ap()
    gg_ap = nc.dram_tensor("gg", [1, H], F32, kind="ExternalInput").ap()
    gd_ap = nc.dram_tensor("gdc", [128, ILK], F32, kind="ExternalInput").ap()
    # [thr_g, m_g/254, thr_d, m_d/254] computed host-side (f64 -> f32)
    ws_ap = nc.dram_tensor("wst", [1, 4], F32, kind="ExternalInput").ap()
    y_ap = nc.dram_tensor("y", [TPC, H], F32, kind="ExternalOutput").ap()
    rg = [list(range(NC_N))]

    with tile.TileContext(nc) as tc:
        with tc.tile_pool(name="dram", bufs=1, space="DRAM") as dram, \
             tc.tile_pool(name="perm", bufs=1) as cp, \
             tc.tile_pool(name="colp", bufs=1) as colp:

            ident_b = cp.tile([128, 128], BF16)
            make_identity(nc, ident_b[:])
            ident_f = cp.tile([128, 128], F32)
            make_identity(nc, ident_f[:])
            ones_b = cp.tile([128, 1], BF16)
            nc.gpsimd.memset(ones_b[:], 1.0)
            epsb = cp.tile([128, 1], F32)
            nc.gpsimd.memset(epsb[:], EPS)
            gdc_sb = cp.tile([128, ILK], F32)
            nc.sync.dma_start(gdc_sb[:], gd_ap[:])
            wst_sb = cp.tile([1, 4], F32)
            nc.sync.dma_start(wst_sb[:], ws_ap[:])

            def bcast_scaled(src, scale, name):
                t1 = colp.tile([1, 1], F32, name=f"{name}_s")
                nc.vector.tensor_scalar_mul(t1[:], src, scale)
                t2 = colp.tile([128, 1], F32, name=f"{name}_b")
                nc.gpsimd.partition_broadcast(t2[:], t1[:])
                return t2

            thr_g = bcast_scaled(wst_sb[0:1, 0:1], 1.0, "thrg")
            nthr_g = bcast_scaled(wst_sb[0:1, 0:1], -1.0, "nthrg")
            c_g = bcast_scaled(wst_sb[0:1, 1:2], 1.0, "cg")      # m_g/254
            thr_d = bcast_scaled(wst_sb[0:1, 2:3], 1.0, "thrd")
            nthr_d = bcast_scaled(wst_sb[0:1, 2:3], -1.0, "nthrd")
            c_d = bcast_scaled(wst_sb[0:1, 3:4], 1.0, "cd")      # m_d/254

            # DRAM scratch
            XR = H + 2  # xq rows + amax hi/lo rows
            xq_sh = dram.tile([XR, 512], BF16)
            xq_full = dram.tile([NC_N * XR, 512], BF16, addr_space="Shared")
            h_dram = dram.tile([IL, TOK], F32)
            sq_ins = [dram.tile([TGH, 512], F32, name=f"sqi{hf}")
                      for hf in range(2)]
            sq_outs = [dram.tile([TGH, 512], F32, addr_space="Shared",
                                 name=f"sqo{hf}") for hf in range(2)]
            am2_ins = [dram.tile([TTH, 128], F32, name=f"a2i{hf}")
                       for hf in range(2)]
            am2_outs = [dram.tile([TTH, 128], F32, addr_space="Shared",
                                  name=f"a2o{hf}") for hf in range(2)]
            qs_rbs = [dram.tile([TTH, 128], F32, name=f"qsr{hf}")
                      for hf in range(2)]
            yps = [dram.tile([TOK // 2, 512], BF16, name=f"yp{n}")
                   for n in range(8)]
            yos = [dram.tile([256, 512], BF16, name=f"yo{n}")
                   for n in range(8)]

            # ---- P1: own-token rmsnorm + int8-grid quant + transpose ----
            amax1 = colp.tile([128, TT], F32)
            with tc.tile_pool(name="xq1", bufs=1) as xq1:
                xqT = xq1.tile([128, HK * 512], BF16)
                g_bc = xq1.tile([128, H], F32)
                gg_sb = xq1.tile([1, H], F32)
                nc.sync.dma_start(gg_sb[:], gg_ap[:])
                nc.gpsimd.partition_broadcast(g_bc[:], gg_sb[:])
                with tc.tile_pool(name="xwork", bufs=2) as xw, \
                     tc.tile_pool(name="psX", bufs=2, space="PSUM") as psX:
                    for t in range(TT):
                        xt = xw.tile([128, H], F32, tag="xt", name=f"xt{t}")
                        nc.sync.dma_start(xt[:],
                                          x_ap[t * 128:(t + 1) * 128, :])
                        xsq = xw.tile([128, H], F32, tag="xsq", name=f"xsq{t}")
                        ssq = colp.tile([128, 1], F32, name=f"ssq{t}")
                        nc.scalar.activation(xsq[:], xt[:], Act.Square,
                                             accum_out=ssq[:])
                        sd = colp.tile([128, 1], F32, name=f"sd{t}")
                        nc.scalar.activation(sd[:], ssq[:], Act.Sqrt,
                                             bias=epsb[:], scale=1.0 / H)
                        rstd = colp.tile([128, 1], F32, name=f"rstd{t}")
                        nc.vector.reciprocal(rstd[:], sd[:])
                        xn = xw.tile([128, H], F32, tag="xn", name=f"xn{t}")
                        nc.vector.tensor_tensor(xn[:], xt[:], g_bc[:],
                                                Alu.mult)
                        nc.vector.tensor_scalar_mul(xn[:], xn[:], rstd[:])
                        am = amax1[:, t:t + 1]
                        nc.vector.tensor_reduce(am, xn[:], axis=X, op=Alu.max,
                                                apply_absolute_value=True)
                        nc.vector.tensor_scalar_max(am, am, EPS)
                        rc = colp.tile([128, 1], F32, name=f"rc{t}")
                        nc.vector.reciprocal(rc[:], am)
                        s1 = colp.tile([128, 1], F32, name=f"s1{t}")
                        nc.vector.tensor_scalar_mul(s1[:], rc[:], 127.0)
                        nc.vector.tensor_scalar(xn[:], xn[:], s1[:], C_MAGIC,
                                                Alu.mult, Alu.add)
                        q = xw.tile([128, H], BF16, tag="q", name=f"q{t}")
                        nc.vector.tensor_scalar(q[:], xn[:], C_MAGIC, None,
                                                Alu.subtract)
                        for i in range(HK):
                            tps = psX.tile([128, 128], BF16, tag="tps",
                                           name=f"tps{t}_{i}")
                            nc.tensor.transpose(
                                tps[:], q[:, i * 128:(i + 1) * 128],
                                ident_b[:])
                            nc.scalar.copy(xqT[:, i * 512 + t * 128:
                                               i * 512 + (t + 1) * 128],
                                           tps[:])
                    # amax cols -> rows; split f32 into bf16 hi+lo and
                    # append as rows H and H+1 of the xq shard
                    psA = psX.tile([TT, 128], F32, tag="psA")
                    nc.tensor.transpose(psA[:], amax1[:], ident_f[:])
                    amr = colp.tile([TT, 128], F32)
                    nc.scalar.copy(amr[:], psA[:])
                    am_hi = xq1.tile([TT, 128], BF16, name="am_hi")
                    nc.vector.tensor_copy(am_hi[:], amr[:])
                    am_hf = xq1.tile([TT, 128], F32, name="am_hf")
                    nc.vector.tensor_copy(am_hf[:], am_hi[:])
                    am_lo = xq1.tile([TT, 128], BF16, name="am_lo")
                    nc.vector.tensor_tensor(am_lo[:], amr[:], am_hf[:],
                                            Alu.subtract)
                    nc.sync.dma_start(
                        xq_sh[H:H + 1, :].rearrange("o (a b) -> (o a) b",
                                                    b=128),
                        am_hi[:])
                    nc.sync.dma_start(
                        xq_sh[H + 1:H + 2, :].rearrange("o (a b) -> (o a) b",
                                                        b=128),
                        am_lo[:])
                nc.sync.dma_start(
                    xq_sh[0:H, :].rearrange("(k p) t -> p k t", p=128),
                    xqT[:].rearrange("p (k t) -> p k t", k=HK))

            nc.gpsimd.collective_compute("AllGather", Alu.bypass,
                                         replica_groups=rg,
                                         ins=[xq_sh[:]], outs=[xq_full[:]])

            # h buffer is DRAM; big SBUF residents below.
            # Stack order: td (lives through mm2) -> q2 tiles -> wg tiles
            # (freed after mm1) -> per-phase pools.
            td_ctx = tc.tile_pool(name="tdp", bufs=1)
            tdp = td_ctx.__enter__()
            td_sb = tdp.tile([128, ILK * H], BF16)
            q2_ctx = tc.tile_pool(name="q2p", bufs=1)
            q2p = q2_ctx.__enter__()
            q2t = [[None] * TGN for _ in range(ILK)]
            for j in range(ILK):
                for tg in range(TGH):
                    q2t[j][tg] = q2p.tile([128, 512], BF16,
                                          name=f"q2_{j}_{tg}")
            # stats/quant pools open below wgt on the pool stack: they
            # outlive mm1 (whose hooks use them) while wgt is popped
            # right after mm1 to free space for mm2-phase pools
            stp_ctx = tc.tile_pool(name="stp", bufs=1)
            stp = stp_ctx.__enter__()
            qp_ctx = tc.tile_pool(name="qp", bufs=2)
            qp = qp_ctx.__enter__()
            wgt_ctx = tc.tile_pool(name="wgt", bufs=1)
            wgp = wgt_ctx.__enter__()
            wg_ts = [wgp.tile([128, HK * 128], BF16, name=f"wgt{b}")
                     for b in range(16)]

            # ---- ternarize (x2 scale): two Sign ops + add ----
            wgv = wg_ap.rearrange("(k p) o -> p k o", p=128)
            with tc.tile_pool(name="ternp", bufs=2) as tp:
                def tern(src_ap, thr, nthr, dst_ap, nm):
                    w = tp.tile([128, HK * 128], F32, tag="tw", name=f"tw{nm}")
                    nc.sync.dma_start(w[:], src_ap)
                    a = tp.tile([128, HK * 128], BF16, tag="ta",
                                name=f"ta{nm}")
                    nc.scalar.activation(a[:], w[:], Act.Sign, bias=nthr[:])
                    bt = tp.tile([128, HK * 128], BF16, tag="tb",
                                 name=f"tb{nm}")
                    nc.scalar.activation(bt[:], w[:], Act.Sign, bias=thr[:])
                    nc.vector.tensor_tensor(dst_ap, a[:], bt[:], Alu.add)

                # col-blocks in mm1 consumption order 0,8,1,9,...
                for jj in range(16):
                    b = (jj // 2) + 8 * (jj % 2)
                    tern(wgv[:, :, b * 128:(b + 1) * 128], thr_g, nthr_g,
                         wg_ts[b][:], f"g{b}")
                for j in range(ILK):
                    tern(wd_ap[j * 128:(j + 1) * 128, :], thr_d, nthr_d,
                         td_sb[:, j * H:(j + 1) * H], f"d{j}")

            def xqv(tg):
                return xq_full[tg * XR:tg * XR + H, :].rearrange(
                    "(k p) t -> p k t", p=128)

            amax2_cols = colp.tile([128, TGN * TTG], F32)

            # ---- mm1: single loop over all 8 token groups ----
            def fire_half_ars(hf, psT1):
                psA2 = psT1.tile([128, 128], F32, tag="pst",
                                 name=f"psa2_{hf}")
                nc.tensor.transpose(
                    psA2[0:TTH, :],
                    amax2_cols[:, hf * TTH:(hf + 1) * TTH],
                    ident_f[:])
                am2r = colp.tile([TTH, 128], F32, name=f"am2r{hf}")
                nc.scalar.copy(am2r[:], psA2[0:TTH, :])
                nc.sync.dma_start(am2_ins[hf][:], am2r[:])
                nc.gpsimd.collective_compute("AllReduce", Alu.add,
                                             replica_groups=rg,
                                             ins=[sq_ins[hf][:]],
                                             outs=[sq_outs[hf][:]])
                nc.gpsimd.collective_compute("AllReduce", Alu.max,
                                             replica_groups=rg,
                                             ins=[am2_ins[hf][:]],
                                             outs=[am2_outs[hf][:]])

            def mm1_all(hooks=None):
                with tc.tile_pool(name="p5", bufs=2) as p5, \
                     tc.tile_pool(name="p5s", bufs=2) as p5s, \
                     tc.tile_pool(name="psM1", bufs=2,
                                  space="PSUM") as psM1, \
                     tc.tile_pool(name="psSq", bufs=1,
                                  space="PSUM") as psSq, \
                     tc.tile_pool(name="psT1", bufs=2,
                                  space="PSUM") as psT1:
                    xqcs, ysbs = {}, {}

                    def issue_tg(tg):
                        t = p5.tile([128, HK * 512], BF16, tag="xqc",
                                    name=f"xqc{tg}")
                        nc.gpsimd.dma_start(
                            t[:].rearrange("p (k t) -> p k t", k=HK),
                            xqv(tg))
                        xqcs[tg] = t
                        hi = p5s.tile([1, 512], BF16, tag="yhi",
                                      name=f"yhi{tg}")
                        nc.gpsimd.dma_start(
                            hi[:], xq_full[tg * XR + H:tg * XR + H + 1, :])
                        lo = p5s.tile([1, 512], BF16, tag="ylo",
                                      name=f"ylo{tg}")
                        nc.gpsimd.dma_start(
                            lo[:],
                            xq_full[tg * XR + H + 1:tg * XR + H + 2, :])
                        yst = p5s.tile([1, 512], F32, tag="rowst",
                                       name=f"yst{tg}")
                        nc.vector.tensor_tensor(yst[:], hi[:], lo[:],
                                                Alu.add)
                        nc.vector.tensor_scalar_mul(yst[:], yst[:],
                                                    c_g[0:1, 0:1])
                        ysb = p5.tile([128, 512], F32, tag="ysb",
                                      name=f"ysb{tg}")
                        nc.gpsimd.partition_broadcast(ysb[:], yst[:])
                        ysbs[tg] = ysb

                    issue_tg(0)
                    for tg in range(TGN):
                        hf = tg // TGH
                        if tg + 1 < TGN:
                            issue_tg(tg + 1)
                        if hooks and tg in hooks:
                            hooks[tg](psSq)
                        xqc = xqcs.pop(tg)
                        ysb = ysbs.pop(tg)
                        acc_am = p5.tile([128, 512], F32, tag="accam",
                                         name=f"accam{tg}")
                        nc.gpsimd.memset(acc_am[:], 0.0)
                        ps_sq = psSq.tile([1, 512], F32, tag="pssq",
                                          name=f"pssq{tg}")
                        for j in range(OLP):
                            pg = psM1.tile([128, 512], F32, tag="pg",
                                           name=f"pg{tg}_{j}")
                            pv = psM1.tile([128, 512], F32, tag="pv",
                                           name=f"pv{tg}_{j}")
                            for k in range(HK):
                                rhs = xqc[:, k * 512:(k + 1) * 512]
                                st, sp = k == 0, k == HK - 1
                                nc.tensor.matmul(
                                    pg[:], wg_ts[j][:, k * 128:(k + 1) * 128],
                                    rhs, start=st, stop=sp)
                                nc.tensor.matmul(
                                    pv[:],
                                    wg_ts[8 + j][:, k * 128:(k + 1) * 128],
                                    rhs, start=st, stop=sp)
                            gs = p5s.tile([128, 512], F32, tag="gs",
                                          name=f"gs{tg}_{j}")
                            nc.vector.tensor_tensor(gs[:], pg[:], ysb[:],
                                                    Alu.mult)
                            sg = gs
                            nc.scalar.activation(sg[:], gs[:], Act.Silu)
                            hj = p5s.tile([128, 512], F32, tag="hj",
                                          name=f"hj{tg}_{j}")
                            nc.vector.tensor_tensor(hj[:], pv[:], ysb[:],
                                                    Alu.mult)
                            nc.vector.tensor_tensor(hj[:], hj[:], sg[:],
                                                    Alu.mult)
                            nc.gpsimd.dma_start(
                                h_dram[j * 128:(j + 1) * 128,
                                       tg * 512:(tg + 1) * 512], hj[:])
                            ha = p5s.tile([128, 512], F32, tag="ha",
                                          name=f"ha{tg}_{j}")
                            nc.scalar.activation(ha[:], hj[:], Act.Abs,
                                                 scale=gdc_sb[:, j:j + 1])
                            nc.vector.tensor_tensor(acc_am[:], acc_am[:],
                                                    ha[:], Alu.max)
                            hsq = p5s.tile([128, 512], BF16, tag="hsq",
                                           name=f"hsq{tg}_{j}")
                            nc.gpsimd.tensor_tensor(hsq[:], hj[:], hj[:],
                                                    Alu.mult)
                            nc.tensor.matmul(ps_sq[:], ones_b[:], hsq[:],
                                             start=(j == 0),
                                             stop=(j == OLP - 1))
                        sqst = p5s.tile([1, 512], F32, tag="rowst",
                                        name=f"sqst{tg}")
                        nc.scalar.copy(sqst[:], ps_sq[:])
                        nc.sync.dma_start(
                            sq_ins[hf][tg - hf * TGH:tg - hf * TGH + 1, :],
                            sqst[:])
                        for t in range(TTG):
                            pst = psT1.tile([128, 128], F32, tag="pst",
                                            name=f"pst{tg}_{t}")
                            nc.tensor.transpose(
                                pst[:], acc_am[:, t * 128:(t + 1) * 128],
                                ident_f[:])
                            nc.vector.tensor_reduce(
                                amax2_cols[:, tg * TTG + t:tg * TTG + t + 1],
                                pst[:], axis=X, op=Alu.max)
                        if tg == TGH - 1:
                            fire_half_ars(0, psT1)
                    fire_half_ars(1, psT1)

            def stats_half(hf, psP, stp):
                sqg = stp.tile([TTH, 128], F32, tag="sqg", name=f"sqg{hf}")
                nc.sync.dma_start(
                    sqg[:],
                    sq_outs[hf][:].rearrange("a (c p) -> (a c) p", p=128))
                amg = stp.tile([TTH, 128], F32, tag="amg", name=f"amg{hf}")
                nc.sync.dma_start(amg[:], am2_outs[hf][:])
                ps1 = psP.tile([128, 128], F32, tag="psp", name=f"ps1{hf}")
                nc.tensor.transpose(ps1[:, 0:TTH], sqg[:],
                                    ident_f[0:TTH, 0:TTH])
                sq_cols = stp.tile([128, TTH], F32, tag="sqc",
                                   name=f"sqc{hf}")
                nc.scalar.copy(sq_cols[:], ps1[:, 0:TTH])
                ps2 = psP.tile([128, 128], F32, tag="psp", name=f"ps2{hf}")
                nc.tensor.transpose(ps2[:, 0:TTH], amg[:],
                                    ident_f[0:TTH, 0:TTH])
                am_cols = stp.tile([128, TTH], F32, tag="amc",
                                   name=f"amc{hf}")
                nc.scalar.copy(am_cols[:], ps2[:, 0:TTH])
                sd2 = stp.tile([128, TTH], F32, tag="sd2", name=f"sd2{hf}")
                nc.scalar.activation(sd2[:], sq_cols[:], Act.Sqrt,
                                     bias=epsb[:], scale=1.0 / I)
                rstd2 = stp.tile([128, TTH], F32, tag="rstd2",
                                 name=f"rstd2{hf}")
                nc.vector.reciprocal(rstd2[:], sd2[:])
                t1c = stp.tile([128, TTH], F32, tag="t1c", name=f"t1c{hf}")
                nc.vector.tensor_tensor(t1c[:], am_cols[:], rstd2[:],
                                        Alu.mult)
                nc.vector.tensor_scalar_max(t1c[:], t1c[:], EPS)
                rc2 = stp.tile([128, TTH], F32, tag="rc2", name=f"rc2{hf}")
                nc.vector.reciprocal(rc2[:], t1c[:])
                s2c = stp.tile([128, TTH], F32, tag="s2c", name=f"s2c{hf}")
                nc.vector.tensor_scalar_mul(s2c[:], rc2[:], 127.0)
                qs_cols = stp.tile([128, TTH], F32, tag="qsc",
                                   name=f"qsc{hf}")
                nc.vector.tensor_tensor(qs_cols[:], rstd2[:], s2c[:],
                                        Alu.mult)
                y2s = colp.tile([128, TTH], F32, name=f"y2s{hf}")
                nc.vector.tensor_scalar(y2s[:], t1c[:], c_d[:], 1.0,
                                        Alu.mult, Alu.mult)
                ps3 = psP.tile([128, 128], F32, tag="psp", name=f"ps3{hf}")
                nc.tensor.transpose(ps3[0:TTH, :], qs_cols[:], ident_f[:])
                qsr = stp.tile([TTH, 128], F32, tag="qsr", name=f"qsr{hf}")
                nc.scalar.copy(qsr[:], ps3[0:TTH, :])
                nc.sync.dma_start(qs_rbs[hf][:], qsr[:])
                return None, y2s

            def quant_half(hf, _unused, qp, tgl):
                for tgi in tgl:
                    tg = hf * TGH + tgi
                    qst = qp.tile([1, 512], F32, tag="qst",
                                  name=f"qst{tg}")
                    nc.sync.dma_start(
                        qst[:],
                        qs_rbs[hf][tgi * TTG:(tgi + 1) * TTG, :].rearrange(
                            "a b -> (a b)").rearrange("(o f) -> o f", o=1))
                    qsb = qp.tile([128, 512], F32, tag="qsb",
                                  name=f"qsb{tg}")
                    nc.gpsimd.partition_broadcast(qsb[:], qst[:])
                    for j in range(ILK):
                        hd = qp.tile([128, 512], F32, tag="hd",
                                     name=f"hd{tg}_{j}")
                        nc.sync.dma_start(
                            hd[:], h_dram[j * 128:(j + 1) * 128,
                                          tg * 512:(tg + 1) * 512])
                        nc.scalar.activation(hd[:], hd[:], Act.Copy,
                                             scale=gdc_sb[:, j:j + 1])
                        nc.vector.tensor_tensor(hd[:], hd[:], qsb[:],
                                                Alu.mult)
                        nc.vector.tensor_scalar(q2t[j][tg][:], hd[:],
                                                C_MAGIC, C_MAGIC,
                                                Alu.add, Alu.subtract)

            def mm2_half(hf, y2s, mp, psM2, hooks=None):
                for hc in range(4):
                    if hooks and hc in hooks:
                        hooks[hc]()
                    for ti in range(TTH):
                        t = hf * TTH + ti
                        tg = t // TTG
                        tloc = (t % TTG) * 128
                        p2 = psM2.tile([128, 512], F32, tag="p2",
                                       name=f"p2_{hf}_{hc}_{ti}")
                        for j in range(ILK):
                            nc.tensor.matmul(
                                p2[:], q2t[j][tg][:, tloc:tloc + 128],
                                td_sb[:, j * H + hc * 512:
                                      j * H + (hc + 1) * 512],
                                start=(j == 0), stop=(j == ILK - 1))
                        yt = mp.tile([128, 512], BF16, tag="yt",
                                     name=f"yt{hf}_{hc}_{ti}")
                        nc.vector.tensor_scalar_mul(yt[:], p2[:],
                                                    y2s[:, ti:ti + 1])
                        nc.sync.dma_start(
                            yps[hf * 4 + hc][ti * 128:(ti + 1) * 128, :],
                            yt[:])
                    nc.gpsimd.collective_compute(
                        "ReduceScatter", Alu.add, replica_groups=rg,
                        ins=[yps[hf * 4 + hc][:]], outs=[yos[hf * 4 + hc][:]])

            hold = {}

            def hook_stats0(psq):
                hold[0] = stats_half(0, psq, stp)

            mm1_all(hooks={
                TGH + 1: hook_stats0,
                TGH + 2: lambda psq: quant_half(0, hold[0][0], qp, (0, 1)),
                TGH + 3: lambda psq: quant_half(0, hold[0][0], qp, (2, 3)),
            })
            wgt_ctx.__exit__(None, None, None)
            psP6_ctx = tc.tile_pool(name="psP6", bufs=1, space="PSUM")
            psP6 = psP6_ctx.__enter__()
            q2b_ctx = tc.tile_pool(name="q2pb", bufs=1)
            q2pb = q2b_ctx.__enter__()
            for j in range(ILK):
                for tg in range(TGH, TGN):
                    q2t[j][tg] = q2pb.tile([128, 512], BF16,
                                           name=f"q2_{j}_{tg}")

            with tc.tile_pool(name="mp", bufs=3) as mp, \
                 tc.tile_pool(name="psM2", bufs=4, space="PSUM") as psM2, \
                 tc.tile_pool(name="p8", bufs=2) as p8:

                def emit_p8(n):
                    hf, hc = n // 4, n % 4
                    yg = p8.tile([128, 2 * 512], BF16, tag="yg",
                                 name=f"yg{n}")
                    nc.sync.dma_start(
                        yg[:].rearrange("p (a b) -> p a b", a=2),
                        yos[n][:].rearrange("(a p) b -> p a b", p=128))
                    yf = p8.tile([128, 2 * 512], F32, tag="yf",
                                 name=f"yf{n}")
                    nc.vector.tensor_copy(yf[:], yg[:])
                    # core's y rows: [0,256) = half0 shard,
                    # [256,512) = half1 shard
                    nc.sync.dma_start(
                        y_ap[hf * 256:(hf + 1) * 256,
                             hc * 512:(hc + 1) * 512].rearrange(
                            "(a p) b -> p a b", p=128),
                        yf[:].rearrange("p (a b) -> p a b", a=2))

                def hook_stats1():
                    hold[1] = stats_half(1, psP6, stp)

                mm2_half(0, hold[0][1], mp, psM2, hooks={
                    1: hook_stats1,
                    2: lambda: quant_half(1, hold[1][0], qp, (0, 1)),
                    3: lambda: quant_half(1, hold[1][0], qp, (2, 3)),
                })
                mm2_half(1, hold[1][1], mp, psM2, hooks={
                    1: lambda: (emit_p8(0), emit_p8(3)),
                    2: lambda: (emit_p8(1), emit_p8(4)),
                    3: lambda: (emit_p8(2), emit_p8(5)),
                })
                for n in range(6, 8):
                    emit_p8(n)

            q2b_ctx.__exit__(None, None, None)
            psP6_ctx.__exit__(None, None, None)
            qp_ctx.__exit__(None, None, None)
            stp_ctx.__exit__(None, None, None)
            q2_ctx.__exit__(None, None, None)
            td_ctx.__exit__(None, None, None)
    return nc


_CACHE = {}


def _get_compiled():
    if "nc" not in _CACHE:
        nc = bacc.Bacc("TRN2", target_bir_lowering=False, debug=False,
                       enable_asserts=False, num_devices=NC_N)
        build(nc)
        nc.compile()
        _CACHE["nc"] = nc
    return _CACHE["nc"]


def make_in_maps(x, w_gate, g_gate, w_down, g_down):
    x2 = np.ascontiguousarray(np.asarray(x, np.float32).reshape(TOK, H))
    wg = np.asarray(w_gate, np.float32)
    wdT = np.asarray(w_down, np.float32).T
    gg = np.ascontiguousarray(np.asarray(g_gate, np.float32).reshape(1, H))
    gd = np.asarray(g_down, np.float32)
    m_g = np.abs(wg.astype(np.float64)).mean()
    m_d = np.abs(np.asarray(w_down, np.float64)).mean()
    m_g = max(m_g, 1e-5)
    m_d = max(m_d, 1e-5)
    wst = np.array([[m_g / 2, m_g / 254, m_d / 2, m_d / 254]],
                   dtype=np.float32)
    in_maps = []
    for c in range(NC_N):
        wgt_c = np.ascontiguousarray(np.concatenate(
            [wg[c * IL:(c + 1) * IL], wg[I + c * IL:I + (c + 1) * IL]],
            axis=0).T)
        wdt_c = np.ascontiguousarray(wdT[c * IL:(c + 1) * IL])
        gdc_c = np.ascontiguousarray(
            gd[c * IL:(c + 1) * IL].reshape(ILK, 128).T)
        in_maps.append({
            "x": x2[c * TPC:(c + 1) * TPC],
            "wgt": wgt_c,
            "wdt": wdt_c,
            "gg": gg,
            "gdc": gdc_c,
            "wst": wst,
        })
    return in_maps


def kernel(x, w_gate, g_gate, w_down, g_down):
    nc = _get_compiled()
    in_maps = make_in_maps(x, w_gate, g_gate, w_down, g_down)
    res = run_bass_kernel_spmd(nc, in_maps, core_ids=list(range(NC_N)))
    # core c holds tokens [c*256,(c+1)*256) of half0 (rows 0..256) and
    # tokens [2048 + c*256, ...) of half1 (rows 256..512)
    out = np.empty((TOK, H), dtype=np.float32)
    for c in range(NC_N):
        yc = res.results[c]["y"]
        out[c * 256:(c + 1) * 256] = yc[:256]
        out[2048 + c * 256:2048 + (c + 1) * 256] = yc[256:]
    return out.reshape(B, S, H).astype(np.float32)


# revision 73
# speedup vs baseline: 1.0456x; 1.0456x over previous
"""HGRNBitMLP (BitNet-style SwiGLU MLP) on 8 TRN2 NeuronCores.

Tensor-parallel: core c owns gate channels [c*1024,(c+1)*1024) and the
matching v channels of w_gate, plus the matching w_down^T rows. Each
core ternarizes only its weight shard (no weight AllGather). The
int8-grid activations xq^T (exact integers in bf16) are AllGathered so
every core streams all 4096 tokens through its channel shard; per-token
amax rides a tiny second AllGather. h is kept f32 (spilled to DRAM).
mm2 partials are scaled per token, cast to bf16, and ReduceScattered
per 512-col H chunk.

Tokens are processed in two 2048-token halves: while half B runs mm1,
half A's tiny stat AllReduces + h-quant complete, so mm2(A) starts the
moment mm1(B) leaves the PE. Output ownership is 2x256 tokens per core;
the host gather reorders.

Ternary weights are materialized as {-2,0,+2} via two Sign activations
(scalar engine) + add, with m/254 folded into the host-computed scale
constants. The weight-quant scalars (threshold, mean|w|) are computed
host-side in f64 and passed as a 4-float input.
"""
import sys

try:
    import concourse  # noqa: F401
except ImportError:
    sys.path.insert(0, "/opt/trn_rl_repo")

import numpy as np

import concourse.tile as tile
from concourse import bacc, mybir
from concourse.bass_utils import run_bass_kernel_spmd
from concourse.masks import make_identity

F32, BF16 = mybir.dt.float32, mybir.dt.bfloat16
Alu = mybir.AluOpType
Act = mybir.ActivationFunctionType
X = mybir.AxisListType.X

NC_N = 8
B, S, H, I = 2, 2048, 2048, 8192
TOK = B * S          # 4096 tokens
TPC = TOK // NC_N    # 512 tokens/core quantized locally
TT = TPC // 128      # 4 token tiles in own shard
HK = H // 128        # 16 contraction tiles for mm1
OL = 2 * I // NC_N   # 2048 local output channels of w_gate (gate+v)
OLP = OL // 2 // 128  # 8 gate/v pair tiles
IL = I // NC_N       # 1024 local intermediate channels
ILK = IL // 128      # 8 contraction tiles for mm2
TGN = TOK // 512     # 8 token groups of 512
TGH = TGN // 2       # 4 token groups per half
TTG = 512 // 128     # 4 t-tiles per group
TTH = 2048 // 128    # 16 t-tiles per half
EPS = 1e-5
C_MAGIC = 12582912.0  # 1.5*2^23; (x+C)-C rounds f32 to nearest-even int


def build(nc):
    x_ap = nc.dram_tensor("x", [TPC, H], F32, kind="ExternalInput").ap()
    wg_ap = nc.dram_tensor("wgt", [H, OL], F32, kind="ExternalInput").ap()
    wd_ap = nc.dram_tensor("wdt", [IL, H], F32, kind="ExternalInput").ap()
    gg_ap = nc.dram_tensor("gg", [1, H], F32, kind="ExternalInput").ap()
    gd_ap = nc.dram_tensor("gdc", [128, ILK], F32, kind="ExternalInput").ap()
    # [thr_g, m_g/254, thr_d, m_d/254] computed host-side (f64 -> f32)
    ws_ap = nc.dram_tensor("wst", [1, 4], F32, kind="ExternalInput").ap()
    y_ap = nc.dram_tensor("y", [TPC, H], F32, kind="ExternalOutput").ap()
    rg = [list(range(NC_N))]

    with tile.TileContext(nc) as tc:
        with tc.tile_pool(name="dram", bufs=1, space="DRAM") as dram, \
             tc.tile_pool(name="perm", bufs=1) as cp, \
             tc.tile_pool(name="colp", bufs=1) as colp:

            ident_b = cp.tile([128, 128], BF16)
            make_identity(nc, ident_b[:])
            ident_f = cp.tile([128, 128], F32)
            make_identity(nc, ident_f[:])
            ones_b = cp.tile([128, 1], BF16)
            nc.gpsimd.memset(ones_b[:], 1.0)
            epsb = cp.tile([128, 1], F32)
            nc.gpsimd.memset(epsb[:], EPS)
            gdc_sb = cp.tile([128, ILK], F32)
            nc.sync.dma_start(gdc_sb[:], gd_ap[:])
            wst_sb = cp.tile([1, 4], F32)
            nc.sync.dma_start(wst_sb[:], ws_ap[:])

            def bcast_scaled(src, scale, name):
                t1 = colp.tile([1, 1], F32, name=f"{name}_s")
                nc.vector.tensor_scalar_mul(t1[:], src, scale)
                t2 = colp.tile([128, 1], F32, name=f"{name}_b")
                nc.gpsimd.partition_broadcast(t2[:], t1[:])
                return t2

            thr_g = bcast_scaled(wst_sb[0:1, 0:1], 1.0, "thrg")
            nthr_g = bcast_scaled(wst_sb[0:1, 0:1], -1.0, "nthrg")
            c_g = bcast_scaled(wst_sb[0:1, 1:2], 1.0, "cg")      # m_g/254
            thr_d = bcast_scaled(wst_sb[0:1, 2:3], 1.0, "thrd")
            nthr_d = bcast_scaled(wst_sb[0:1, 2:3], -1.0, "nthrd")
            c_d = bcast_scaled(wst_sb[0:1, 3:4], 1.0, "cd")      # m_d/254

            # DRAM scratch
            XR = H + 2  # xq rows + amax hi/lo rows
            xq_sh = dram.tile([XR, 512], BF16)
            xq_full = dram.tile([NC_N * XR, 512], BF16, addr_space="Shared")
            h_dram = dram.tile([IL, TOK], F32)
            sq_ins = [dram.tile([TGH, 512], F32, name=f"sqi{hf}")
                      for hf in range(2)]
            sq_outs = [dram.tile([TGH, 512], F32, addr_space="Shared",
                                 name=f"sqo{hf}") for hf in range(2)]
            am2_ins = [dram.tile([TTH, 128], F32, name=f"a2i{hf}")
                       for hf in range(2)]
            am2_outs = [dram.tile([TTH, 128], F32, addr_space="Shared",
                                  name=f"a2o{hf}") for hf in range(2)]
            qs_rbs = [dram.tile([TTH, 128], F32, name=f"qsr{hf}")
                      for hf in range(2)]
            yps = [dram.tile([TOK // 2, 512], BF16, name=f"yp{n}")
                   for n in range(8)]
            yos = [dram.tile([256, 512], BF16, name=f"yo{n}")
                   for n in range(8)]

            # ---- P1: own-token rmsnorm + int8-grid quant + transpose ----
            amax1 = colp.tile([128, TT], F32)
            with tc.tile_pool(name="xq1", bufs=1) as xq1:
                xqT = xq1.tile([128, HK * 512], BF16)
                g_bc = xq1.tile([128, H], F32)
                gg_sb = xq1.tile([1, H], F32)
                nc.sync.dma_start(gg_sb[:], gg_ap[:])
                nc.gpsimd.partition_broadcast(g_bc[:], gg_sb[:])
                with tc.tile_pool(name="xwork", bufs=2) as xw, \
                     tc.tile_pool(name="psX", bufs=2, space="PSUM") as psX:
                    for t in range(TT):
                        xt = xw.tile([128, H], F32, tag="xt", name=f"xt{t}")
                        nc.sync.dma_start(xt[:],
                                          x_ap[t * 128:(t + 1) * 128, :])
                        xsq = xw.tile([128, H], F32, tag="xsq", name=f"xsq{t}")
                        ssq = colp.tile([128, 1], F32, name=f"ssq{t}")
                        nc.scalar.activation(xsq[:], xt[:], Act.Square,
                                             accum_out=ssq[:])
                        sd = colp.tile([128, 1], F32, name=f"sd{t}")
                        nc.scalar.activation(sd[:], ssq[:], Act.Sqrt,
                                             bias=epsb[:], scale=1.0 / H)
                        rstd = colp.tile([128, 1], F32, name=f"rstd{t}")
                        nc.vector.reciprocal(rstd[:], sd[:])
                        xn = xw.tile([128, H], F32, tag="xn", name=f"xn{t}")
                        nc.vector.tensor_tensor(xn[:], xt[:], g_bc[:],
                                                Alu.mult)
                        nc.vector.tensor_scalar_mul(xn[:], xn[:], rstd[:])
                        am = amax1[:, t:t + 1]
                        nc.vector.tensor_reduce(am, xn[:], axis=X, op=Alu.max,
                                                apply_absolute_value=True)
                        nc.vector.tensor_scalar_max(am, am, EPS)
                        rc = colp.tile([128, 1], F32, name=f"rc{t}")
                        nc.vector.reciprocal(rc[:], am)
                        s1 = colp.tile([128, 1], F32, name=f"s1{t}")
                        nc.vector.tensor_scalar_mul(s1[:], rc[:], 127.0)
                        nc.vector.tensor_scalar(xn[:], xn[:], s1[:], C_MAGIC,
                                                Alu.mult, Alu.add)
                        q = xw.tile([128, H], BF16, tag="q", name=f"q{t}")
                        nc.vector.tensor_scalar(q[:], xn[:], C_MAGIC, None,
                                                Alu.subtract)
                        for i in range(HK):
                            tps = psX.tile([128, 128], BF16, tag="tps",
                                           name=f"tps{t}_{i}")
                            nc.tensor.transpose(
                                tps[:], q[:, i * 128:(i + 1) * 128],
                                ident_b[:])
                            nc.scalar.copy(xqT[:, i * 512 + t * 128:
                                               i * 512 + (t + 1) * 128],
                                           tps[:])
                    # amax cols -> rows; split f32 into bf16 hi+lo and
                    # append as rows H and H+1 of the xq shard
                    psA = psX.tile([TT, 128], F32, tag="psA")
                    nc.tensor.transpose(psA[:], amax1[:], ident_f[:])
                    amr = colp.tile([TT, 128], F32)
                    nc.scalar.copy(amr[:], psA[:])
                    am_hi = xq1.tile([TT, 128], BF16, name="am_hi")
                    nc.vector.tensor_copy(am_hi[:], amr[:])
                    am_hf = xq1.tile([TT, 128], F32, name="am_hf")
                    nc.vector.tensor_copy(am_hf[:], am_hi[:])
                    am_lo = xq1.tile([TT, 128], BF16, name="am_lo")
                    nc.vector.tensor_tensor(am_lo[:], amr[:], am_hf[:],
                                            Alu.subtract)
                    nc.sync.dma_start(
                        xq_sh[H:H + 1, :].rearrange("o (a b) -> (o a) b",
                                                    b=128),
                        am_hi[:])
                    nc.sync.dma_start(
                        xq_sh[H + 1:H + 2, :].rearrange("o (a b) -> (o a) b",
                                                        b=128),
                        am_lo[:])
                nc.sync.dma_start(
                    xq_sh[0:H, :].rearrange("(k p) t -> p k t", p=128),
                    xqT[:].rearrange("p (k t) -> p k t", k=HK))

            nc.gpsimd.collective_compute("AllGather", Alu.bypass,
                                         replica_groups=rg,
                                         ins=[xq_sh[:]], outs=[xq_full[:]])

            # h buffer is DRAM; big SBUF residents below.
            # Stack order: td (lives through mm2) -> q2 tiles -> wg tiles
            # (freed after mm1) -> per-phase pools.
            td_ctx = tc.tile_pool(name="tdp", bufs=1)
            tdp = td_ctx.__enter__()
            td_sb = tdp.tile([128, ILK * H], BF16)
            q2_ctx = tc.tile_pool(name="q2p", bufs=1)
            q2p = q2_ctx.__enter__()
            q2t = [[None] * TGN for _ in range(ILK)]
            for j in range(ILK):
                for tg in range(TGH):
                    q2t[j][tg] = q2p.tile([128, 512], BF16,
                                          name=f"q2_{j}_{tg}")
            # stats/quant pools open below wgt on the pool stack: they
            # outlive mm1 (whose hooks use them) while wgt is popped
            # right after mm1 to free space for mm2-phase pools
            stp_ctx = tc.tile_pool(name="stp", bufs=1)
            stp = stp_ctx.__enter__()
            qp_ctx = tc.tile_pool(name="qp", bufs=2)
            qp = qp_ctx.__enter__()
            wgt_ctx = tc.tile_pool(name="wgt", bufs=1)
            wgp = wgt_ctx.__enter__()
            wg_ts = [wgp.tile([128, HK * 128], BF16, name=f"wgt{b}")
                     for b in range(16)]

            # ---- ternarize (x2 scale): two Sign ops + add ----
            wgv = wg_ap.rearrange("(k p) o -> p k o", p=128)
            with tc.tile_pool(name="ternp", bufs=2) as tp:
                def tern(src_ap, thr, nthr, dst_ap, nm):
                    w = tp.tile([128, HK * 128], F32, tag="tw",
                                name=f"tw{nm}", bufs=5)
                    nc.sync.dma_start(w[:], src_ap)
                    a = tp.tile([128, HK * 128], BF16, tag="ta",
                                name=f"ta{nm}")
                    nc.scalar.activation(a[:], w[:], Act.Sign, bias=nthr[:])
                    bt = tp.tile([128, HK * 128], BF16, tag="tb",
                                 name=f"tb{nm}")
                    nc.scalar.activation(bt[:], w[:], Act.Sign, bias=thr[:])
                    nc.vector.tensor_tensor(dst_ap, a[:], bt[:], Alu.add)

                # col-blocks in mm1 consumption order 0,8,1,9,...
                for jj in range(16):
                    b = (jj // 2) + 8 * (jj % 2)
                    tern(wgv[:, :, b * 128:(b + 1) * 128], thr_g, nthr_g,
                         wg_ts[b][:], f"g{b}")
                for j in range(ILK):
                    tern(wd_ap[j * 128:(j + 1) * 128, :], thr_d, nthr_d,
                         td_sb[:, j * H:(j + 1) * H], f"d{j}")

            def xqv(tg):
                return xq_full[tg * XR:tg * XR + H, :].rearrange(
                    "(k p) t -> p k t", p=128)

            amax2_cols = colp.tile([128, TGN * TTG], F32)

            # ---- mm1: single loop over all 8 token groups ----
            def fire_half_ars(hf, psT1):
                psA2 = psT1.tile([128, 128], F32, tag="pst",
                                 name=f"psa2_{hf}")
                nc.tensor.transpose(
                    psA2[0:TTH, :],
                    amax2_cols[:, hf * TTH:(hf + 1) * TTH],
                    ident_f[:])
                am2r = colp.tile([TTH, 128], F32, name=f"am2r{hf}")
                nc.scalar.copy(am2r[:], psA2[0:TTH, :])
                nc.sync.dma_start(am2_ins[hf][:], am2r[:])
                nc.gpsimd.collective_compute("AllReduce", Alu.add,
                                             replica_groups=rg,
                                             ins=[sq_ins[hf][:]],
                                             outs=[sq_outs[hf][:]])
                nc.gpsimd.collective_compute("AllReduce", Alu.max,
                                             replica_groups=rg,
                                             ins=[am2_ins[hf][:]],
                                             outs=[am2_outs[hf][:]])

            def mm1_all(hooks=None):
                with tc.tile_pool(name="p5", bufs=2) as p5, \
                     tc.tile_pool(name="p5s", bufs=2) as p5s, \
                     tc.tile_pool(name="psM1", bufs=2,
                                  space="PSUM") as psM1, \
                     tc.tile_pool(name="psSq", bufs=1,
                                  space="PSUM") as psSq, \
                     tc.tile_pool(name="psT1", bufs=2,
                                  space="PSUM") as psT1:
                    xqcs, ysbs = {}, {}

                    def issue_tg(tg):
                        t = p5.tile([128, HK * 512], BF16, tag="xqc",
                                    name=f"xqc{tg}")
                        nc.gpsimd.dma_start(
                            t[:].rearrange("p (k t) -> p k t", k=HK),
                            xqv(tg))
                        xqcs[tg] = t
                        hi = p5s.tile([1, 512], BF16, tag="yhi",
                                      name=f"yhi{tg}")
                        nc.gpsimd.dma_start(
                            hi[:], xq_full[tg * XR + H:tg * XR + H + 1, :])
                        lo = p5s.tile([1, 512], BF16, tag="ylo",
                                      name=f"ylo{tg}")
                        nc.gpsimd.dma_start(
                            lo[:],
                            xq_full[tg * XR + H + 1:tg * XR + H + 2, :])
                        yst = p5s.tile([1, 512], F32, tag="rowst",
                                       name=f"yst{tg}")
                        nc.vector.tensor_tensor(yst[:], hi[:], lo[:],
                                                Alu.add)
                        nc.vector.tensor_scalar_mul(yst[:], yst[:],
                                                    c_g[0:1, 0:1])
                        ysb = p5.tile([128, 512], F32, tag="ysb",
                                      name=f"ysb{tg}")
                        nc.gpsimd.partition_broadcast(ysb[:], yst[:])
                        ysbs[tg] = ysb

                    issue_tg(0)
                    for tg in range(TGN):
                        hf = tg // TGH
                        if tg + 1 < TGN:
                            issue_tg(tg + 1)
                        if hooks and tg in hooks:
                            hooks[tg](psSq)
                        xqc = xqcs.pop(tg)
                        ysb = ysbs.pop(tg)
                        acc_am = p5.tile([128, 512], F32, tag="accam",
                                         name=f"accam{tg}")
                        nc.gpsimd.memset(acc_am[:], 0.0)
                        ps_sq = psSq.tile([1, 512], F32, tag="pssq",
                                          name=f"pssq{tg}")
                        for j in range(OLP):
                            pg = psM1.tile([128, 512], F32, tag="pg",
                                           name=f"pg{tg}_{j}")
                            pv = psM1.tile([128, 512], F32, tag="pv",
                                           name=f"pv{tg}_{j}")
                            for k in range(HK):
                                rhs = xqc[:, k * 512:(k + 1) * 512]
                                st, sp = k == 0, k == HK - 1
                                nc.tensor.matmul(
                                    pg[:], wg_ts[j][:, k * 128:(k + 1) * 128],
                                    rhs, start=st, stop=sp)
                                nc.tensor.matmul(
                                    pv[:],
                                    wg_ts[8 + j][:, k * 128:(k + 1) * 128],
                                    rhs, start=st, stop=sp)
                            gs = p5s.tile([128, 512], F32, tag="gs",
                                          name=f"gs{tg}_{j}")
                            nc.vector.tensor_tensor(gs[:], pg[:], ysb[:],
                                                    Alu.mult)
                            sg = gs
                            nc.scalar.activation(sg[:], gs[:], Act.Silu)
                            hj = p5s.tile([128, 512], F32, tag="hj",
                                          name=f"hj{tg}_{j}")
                            nc.vector.tensor_tensor(hj[:], pv[:], ysb[:],
                                                    Alu.mult)
                            nc.vector.tensor_tensor(hj[:], hj[:], sg[:],
                                                    Alu.mult)
                            nc.gpsimd.dma_start(
                                h_dram[j * 128:(j + 1) * 128,
                                       tg * 512:(tg + 1) * 512], hj[:])
                            ha = p5s.tile([128, 512], F32, tag="ha",
                                          name=f"ha{tg}_{j}")
                            nc.scalar.activation(ha[:], hj[:], Act.Abs,
                                                 scale=gdc_sb[:, j:j + 1])
                            nc.vector.tensor_tensor(acc_am[:], acc_am[:],
                                                    ha[:], Alu.max)
                            hsq = p5s.tile([128, 512], BF16, tag="hsq",
                                           name=f"hsq{tg}_{j}")
                            nc.gpsimd.tensor_tensor(hsq[:], hj[:], hj[:],
                                                    Alu.mult)
                            nc.tensor.matmul(ps_sq[:], ones_b[:], hsq[:],
                                             start=(j == 0),
                                             stop=(j == OLP - 1))
                        sqst = p5s.tile([1, 512], F32, tag="rowst",
                                        name=f"sqst{tg}")
                        nc.scalar.copy(sqst[:], ps_sq[:])
                        nc.sync.dma_start(
                            sq_ins[hf][tg - hf * TGH:tg - hf * TGH + 1, :],
                            sqst[:])
                        for t in range(TTG):
                            pst = psT1.tile([128, 128], F32, tag="pst",
                                            name=f"pst{tg}_{t}")
                            nc.tensor.transpose(
                                pst[:], acc_am[:, t * 128:(t + 1) * 128],
                                ident_f[:])
                            nc.vector.tensor_reduce(
                                amax2_cols[:, tg * TTG + t:tg * TTG + t + 1],
                                pst[:], axis=X, op=Alu.max)
                        if tg == TGH - 1:
                            fire_half_ars(0, psT1)
                    fire_half_ars(1, psT1)

            def stats_half(hf, psP, stp):
                sqg = stp.tile([TTH, 128], F32, tag="sqg", name=f"sqg{hf}")
                nc.sync.dma_start(
                    sqg[:],
                    sq_outs[hf][:].rearrange("a (c p) -> (a c) p", p=128))
                amg = stp.tile([TTH, 128], F32, tag="amg", name=f"amg{hf}")
                nc.sync.dma_start(amg[:], am2_outs[hf][:])
                ps1 = psP.tile([128, 128], F32, tag="psp", name=f"ps1{hf}")
                nc.tensor.transpose(ps1[:, 0:TTH], sqg[:],
                                    ident_f[0:TTH, 0:TTH])
                sq_cols = stp.tile([128, TTH], F32, tag="sqc",
                                   name=f"sqc{hf}")
                nc.scalar.copy(sq_cols[:], ps1[:, 0:TTH])
                ps2 = psP.tile([128, 128], F32, tag="psp", name=f"ps2{hf}")
                nc.tensor.transpose(ps2[:, 0:TTH], amg[:],
                                    ident_f[0:TTH, 0:TTH])
                am_cols = stp.tile([128, TTH], F32, tag="amc",
                                   name=f"amc{hf}")
                nc.scalar.copy(am_cols[:], ps2[:, 0:TTH])
                sd2 = stp.tile([128, TTH], F32, tag="sd2", name=f"sd2{hf}")
                nc.scalar.activation(sd2[:], sq_cols[:], Act.Sqrt,
                                     bias=epsb[:], scale=1.0 / I)
                rstd2 = stp.tile([128, TTH], F32, tag="rstd2",
                                 name=f"rstd2{hf}")
                nc.vector.reciprocal(rstd2[:], sd2[:])
                t1c = stp.tile([128, TTH], F32, tag="t1c", name=f"t1c{hf}")
                nc.vector.tensor_tensor(t1c[:], am_cols[:], rstd2[:],
                                        Alu.mult)
                nc.vector.tensor_scalar_max(t1c[:], t1c[:], EPS)
                rc2 = stp.tile([128, TTH], F32, tag="rc2", name=f"rc2{hf}")
                nc.vector.reciprocal(rc2[:], t1c[:])
                s2c = stp.tile([128, TTH], F32, tag="s2c", name=f"s2c{hf}")
                nc.vector.tensor_scalar_mul(s2c[:], rc2[:], 127.0)
                qs_cols = stp.tile([128, TTH], F32, tag="qsc",
                                   name=f"qsc{hf}")
                nc.vector.tensor_tensor(qs_cols[:], rstd2[:], s2c[:],
                                        Alu.mult)
                y2s = colp.tile([128, TTH], F32, name=f"y2s{hf}")
                nc.vector.tensor_scalar(y2s[:], t1c[:], c_d[:], 1.0,
                                        Alu.mult, Alu.mult)
                ps3 = psP.tile([128, 128], F32, tag="psp", name=f"ps3{hf}")
                nc.tensor.transpose(ps3[0:TTH, :], qs_cols[:], ident_f[:])
                qsr = stp.tile([TTH, 128], F32, tag="qsr", name=f"qsr{hf}")
                nc.scalar.copy(qsr[:], ps3[0:TTH, :])
                nc.sync.dma_start(qs_rbs[hf][:], qsr[:])
                return None, y2s

            def quant_half(hf, _unused, qp, tgl):
                for tgi in tgl:
                    tg = hf * TGH + tgi
                    qst = qp.tile([1, 512], F32, tag="qst",
                                  name=f"qst{tg}")
                    nc.sync.dma_start(
                        qst[:],
                        qs_rbs[hf][tgi * TTG:(tgi + 1) * TTG, :].rearrange(
                            "a b -> (a b)").rearrange("(o f) -> o f", o=1))
                    qsb = qp.tile([128, 512], F32, tag="qsb",
                                  name=f"qsb{tg}")
                    nc.gpsimd.partition_broadcast(qsb[:], qst[:])
                    for j in range(ILK):
                        hd = qp.tile([128, 512], F32, tag="hd",
                                     name=f"hd{tg}_{j}")
                        nc.sync.dma_start(
                            hd[:], h_dram[j * 128:(j + 1) * 128,
                                          tg * 512:(tg + 1) * 512])
                        nc.scalar.activation(hd[:], hd[:], Act.Copy,
                                             scale=gdc_sb[:, j:j + 1])
                        nc.vector.tensor_tensor(hd[:], hd[:], qsb[:],
                                                Alu.mult)
                        nc.vector.tensor_scalar(q2t[j][tg][:], hd[:],
                                                C_MAGIC, C_MAGIC,
                                                Alu.add, Alu.subtract)

            def mm2_half(hf, y2s, mp, psM2, hooks=None):
                for hc in range(4):
                    if hooks and hc in hooks:
                        hooks[hc]()
                    for ti in range(TTH):
                        t = hf * TTH + ti
                        tg = t // TTG
                        tloc = (t % TTG) * 128
                        p2 = psM2.tile([128, 512], F32, tag="p2",
                                       name=f"p2_{hf}_{hc}_{ti}")
                        for j in range(ILK):
                            nc.tensor.matmul(
                                p2[:], q2t[j][tg][:, tloc:tloc + 128],
                                td_sb[:, j * H + hc * 512:
                                      j * H + (hc + 1) * 512],
                                start=(j == 0), stop=(j == ILK - 1))
                        yt = mp.tile([128, 512], BF16, tag="yt",
                                     name=f"yt{hf}_{hc}_{ti}")
                        nc.vector.tensor_scalar_mul(yt[:], p2[:],
                                                    y2s[:, ti:ti + 1])
                        nc.sync.dma_start(
                            yps[hf * 4 + hc][ti * 128:(ti + 1) * 128, :],
                            yt[:])
                    nc.gpsimd.collective_compute(
                        "ReduceScatter", Alu.add, replica_groups=rg,
                        ins=[yps[hf * 4 + hc][:]], outs=[yos[hf * 4 + hc][:]])

            hold = {}

            def hook_stats0(psq):
                hold[0] = stats_half(0, psq, stp)

            mm1_all(hooks={
                TGH + 1: hook_stats0,
                TGH + 2: lambda psq: quant_half(0, hold[0][0], qp, (0, 1)),
                TGH + 3: lambda psq: quant_half(0, hold[0][0], qp, (2, 3)),
            })
            wgt_ctx.__exit__(None, None, None)
            psP6_ctx = tc.tile_pool(name="psP6", bufs=1, space="PSUM")
            psP6 = psP6_ctx.__enter__()
            q2b_ctx = tc.tile_pool(name="q2pb", bufs=1)
            q2pb = q2b_ctx.__enter__()
            for j in range(ILK):
                for tg in range(TGH, TGN):
                    q2t[j][tg] = q2pb.tile([128, 512], BF16,
                                           name=f"q2_{j}_{tg}")

            with tc.tile_pool(name="mp", bufs=3) as mp, \
                 tc.tile_pool(name="psM2", bufs=4, space="PSUM") as psM2, \
                 tc.tile_pool(name="p8", bufs=2) as p8:

                def emit_p8(n):
                    hf, hc = n // 4, n % 4
                    yg = p8.tile([128, 2 * 512], BF16, tag="yg",
                                 name=f"yg{n}")
                    nc.sync.dma_start(
                        yg[:].rearrange("p (a b) -> p a b", a=2),
                        yos[n][:].rearrange("(a p) b -> p a b", p=128))
                    yf = p8.tile([128, 2 * 512], F32, tag="yf",
                                 name=f"yf{n}")
                    nc.vector.tensor_copy(yf[:], yg[:])
                    # core's y rows: [0,256) = half0 shard,
                    # [256,512) = half1 shard
                    nc.sync.dma_start(
                        y_ap[hf * 256:(hf + 1) * 256,
                             hc * 512:(hc + 1) * 512].rearrange(
                            "(a p) b -> p a b", p=128),
                        yf[:].rearrange("p (a b) -> p a b", a=2))

                def hook_stats1():
                    hold[1] = stats_half(1, psP6, stp)

                mm2_half(0, hold[0][1], mp, psM2, hooks={
                    1: hook_stats1,
                    2: lambda: quant_half(1, hold[1][0], qp, (0, 1)),
                    3: lambda: quant_half(1, hold[1][0], qp, (2, 3)),
                })
                mm2_half(1, hold[1][1], mp, psM2, hooks={
                    1: lambda: (emit_p8(0), emit_p8(3)),
                    2: lambda: (emit_p8(1), emit_p8(4)),
                    3: lambda: (emit_p8(2), emit_p8(5)),
                })
                for n in range(6, 8):
                    emit_p8(n)

            q2b_ctx.__exit__(None, None, None)
            psP6_ctx.__exit__(None, None, None)
            qp_ctx.__exit__(None, None, None)
            stp_ctx.__exit__(None, None, None)
            q2_ctx.__exit__(None, None, None)
            td_ctx.__exit__(None, None, None)
    return nc


_CACHE = {}


def _get_compiled():
    if "nc" not in _CACHE:
        nc = bacc.Bacc("TRN2", target_bir_lowering=False, debug=False,
                       enable_asserts=False, num_devices=NC_N)
        build(nc)
        nc.compile()
        _CACHE["nc"] = nc
    return _CACHE["nc"]


def make_in_maps(x, w_gate, g_gate, w_down, g_down):
    x2 = np.ascontiguousarray(np.asarray(x, np.float32).reshape(TOK, H))
    wg = np.asarray(w_gate, np.float32)
    wdT = np.asarray(w_down, np.float32).T
    gg = np.ascontiguousarray(np.asarray(g_gate, np.float32).reshape(1, H))
    gd = np.asarray(g_down, np.float32)
    m_g = np.abs(wg.astype(np.float64)).mean()
    m_d = np.abs(np.asarray(w_down, np.float64)).mean()
    m_g = max(m_g, 1e-5)
    m_d = max(m_d, 1e-5)
    wst = np.array([[m_g / 2, m_g / 254, m_d / 2, m_d / 254]],
                   dtype=np.float32)
    in_maps = []
    for c in range(NC_N):
        wgt_c = np.ascontiguousarray(np.concatenate(
            [wg[c * IL:(c + 1) * IL], wg[I + c * IL:I + (c + 1) * IL]],
            axis=0).T)
        wdt_c = np.ascontiguousarray(wdT[c * IL:(c + 1) * IL])
        gdc_c = np.ascontiguousarray(
            gd[c * IL:(c + 1) * IL].reshape(ILK, 128).T)
        in_maps.append({
            "x": x2[c * TPC:(c + 1) * TPC],
            "wgt": wgt_c,
            "wdt": wdt_c,
            "gg": gg,
            "gdc": gdc_c,
            "wst": wst,
        })
    return in_maps


def kernel(x, w_gate, g_gate, w_down, g_down):
    nc = _get_compiled()
    in_maps = make_in_maps(x, w_gate, g_gate, w_down, g_down)
    res = run_bass_kernel_spmd(nc, in_maps, core_ids=list(range(NC_N)))
    # core c holds tokens [c*256,(c+1)*256) of half0 (rows 0..256) and
    # tokens [2048 + c*256, ...) of half1 (rows 256..512)
    out = np.empty((TOK, H), dtype=np.float32)
    for c in range(NC_N):
        yc = res.results[c]["y"]
        out[c * 256:(c + 1) * 256] = yc[:256]
        out[2048 + c * 256:2048 + (c + 1) * 256] = yc[256:]
    return out.reshape(B, S, H).astype(np.float32)
